# revision 37
# baseline (speedup 1.0000x reference)
"""Multi-head attention Bass/Tile kernel for Trainium2, SPMD over 8 NeuronCores.

Sharding (tensor-parallel over heads, per-batch): core c handles batch
b = c//2 and head half hh2 = c%2 (8 of 16 heads), for ALL 2048 queries.
Each core projects K/Q/V only for its own heads, runs full attention for
those heads, and computes a PARTIAL output projection
out_c = mh_own @ Wo_own^T. The host sums the two partials of a core pair
(and adds bo) — no cross-core collective needed.

Key-compaction: masked keys contribute exactly 0 to softmax (their exp
underflows to 0 under the -1e4 bias), so the host gathers only the unmasked
key columns of x (padded to a 128-multiple; padding carries mask=0 so it is
biased away identically). K/V projection, scores, exp and av then run on
~half the key tiles. Queries are untouched. This is numerically identical
to full-width attention.

Other layout tricks:
  - keys live on the PSUM partition axis (scores are computed transposed,
    S^T[k, q]), so the attention-mask bias is a per-partition bias fused
    into the ACT exp, and the av matmul (contract over k) needs no on-chip
    transposes;
  - softmax denominators come from a ones-column appended to V (row 64 of
    the z accumulator); the reciprocal runs straight off PSUM row 64 with a
    bf16 output (wide exponent — no rescale), and a PE ones-matmul
    broadcasts it across 64 partitions;
  - no max-subtraction: scaled scores are O(+-10), exp fits fp16/fp32.
All matmuls run with fp16 operands (1 cycle/row on the PE) accumulating in
fp32 PSUM; x and the weights are pre-cast to fp16 on the host so the device
DMAs half the bytes and does no cast work. Partial outputs are written as
fp16 (host upcasts and sums).
"""
import sys

import numpy as np

sys.path.insert(0, "/opt/trn_rl_repo")

import concourse.bacc as bacc
import concourse.mybir as mybir
import concourse.tile as tile
from concourse import bass_utils
from concourse._compat import get_trn_type
from concourse.bass import ts

P = 128
S = 2048
D = 1024
NH = 16           # total heads
NHL = 8           # heads per core
NHPL = 4          # head pairs per core
HEL = NHL * 64    # 512 head-embed dims per core
KT_FULL = S // P  # 16 key tiles uncompacted
DT = D // P       # 8 contraction tiles over d_model
SCALE = 0.125     # 1/sqrt(64)
F32 = mybir.dt.float32
F16 = mybir.dt.float16
BF16 = mybir.dt.bfloat16
I32 = mybir.dt.int32
MUL = mybir.AluOpType.mult
ADD = mybir.AluOpType.add
Exp = mybir.ActivationFunctionType.Exp

N_CORES = 8


def build_nc(stage=4, loop_n=None, ebufs=6, zero_bias=False, ktp=KT_FULL,
             avsplit=False, tiny_exp=False, no_in_dma=False,
             no_out_dma=False, use_bf16=False, padk=True, padrep=True,
             depth2=True):
    F16 = BF16 if use_bf16 else mybir.dt.float16  # shadows module constant
    SCB = 2 if depth2 else 3  # sc-tag PSUM bufs (depth2 needs 4 z banks)
    SK = ktp * P          # compacted+padded key count
    nc = bacc.Bacc(get_trn_type() or "TRN2", target_bir_lowering=False, debug=False)

    xT = nc.dram_tensor("xT", [D, S], F16, kind="ExternalInput")
    xkT = nc.dram_tensor("xkT", [D, SK], F16, kind="ExternalInput")
    wqT = nc.dram_tensor("wqT", [D, HEL], F16, kind="ExternalInput")
    wkT = nc.dram_tensor("wkT", [D, HEL], F16, kind="ExternalInput")
    wvT = nc.dram_tensor("wvT", [D, HEL], F16, kind="ExternalInput")
    woT = nc.dram_tensor("woT", [HEL, D], F16, kind="ExternalInput")
    mask = nc.dram_tensor("mask", [SK], I32, kind="ExternalInput")
    if not zero_bias:
        bq = nc.dram_tensor("bq", [HEL], F32, kind="ExternalInput")
        bk = nc.dram_tensor("bk", [HEL], F32, kind="ExternalInput")
        bv = nc.dram_tensor("bv", [HEL], F32, kind="ExternalInput")
    out = nc.dram_tensor("out", [S, D], F16, kind="ExternalOutput")

    xT_t = xT.rearrange("(dt p) s -> p dt s", p=P)          # [128, 8, 2048]
    xkT_t = xkT.rearrange("(dt p) s -> p dt s", p=P)        # [128, 8, SK]
    wqT_t = wqT.rearrange("(dt p) he -> p dt he", p=P)      # [128, 8, 512]
    wkT_t = wkT.rearrange("(dt p) he -> p dt he", p=P)
    wvT_t = wvT.rearrange("(dt p) he -> p dt he", p=P)
    woT_t = woT.rearrange("(hp p) d -> p hp d", p=P)        # [128, 4, 1024]
    mask_t = mask.rearrange("(t p) -> p t", p=P)            # [128, ktp]
    out_t = out.rearrange("(qt p) d -> p qt d", p=P)        # [128, 16, 1024]

    import contextlib

    with tile.TileContext(nc) as tc:
        with (
            tc.For_i(0, loop_n, 1) if loop_n else contextlib.nullcontext(),
            tc.tile_pool(name="const", bufs=1) as cpool,
            tc.tile_pool(name="persist", bufs=1) as big,
            tc.tile_pool(name="psum", bufs=1, space="PSUM") as pps,
        ):
            # ---- persistent fp16 operand tiles, DMA'd directly (host casts).
            # Load order follows first use: wk, compacted keys-x, wq, the
            # queries-x, wv, wo.
            wk16 = big.tile([P, DT, HEL], F16)
            wv16 = big.tile([P, DT, HEL], F16)
            wq16 = big.tile([P, DT, HEL], F16)
            wo16 = big.tile([P, NHPL, D], F16)
            xk16 = big.tile([P, DT, SK], F16)                # keys/values x^T
            x16 = big.tile([P, DT, S], F16)                  # queries x^T
            if padk:
                # K^T zero-padded to full 128 contraction rows per head-half
                # (K<=64 matmuls run the PE at half rate; zero rows restore
                # K=128 at full rate and contribute nothing). The zero halves
                # are written per-chunk inside kproj_chunk (Pool engine) so
                # they carry the same WAR deps as the real evacuation.
                kTz = big.tile([P, 2, NHPL, SK], F16)
            else:
                kT16 = big.tile([P, NHPL, SK], F16)          # K^T [he, k]
            v16 = big.tile([P, ktp, NHL * 65], F16)          # V rows + ones col
            mh16 = big.tile([P, NHPL, S], F16)               # mh^T [he, q]

            xsplit = min(512, SK)
            if no_in_dma:
                # timing ablation: back the input tiles with memsets so the
                # scheduler sees writers, without any HBM traffic
                nc.vector.memset(x16[:], 0.125)
                nc.vector.memset(wk16[:], 0.125)
                nc.vector.memset(wv16[:], 0.125)
                nc.vector.memset(wq16[:], 0.125)
                nc.vector.memset(wo16[:], 0.125)
                nc.gpsimd.memset(xk16[:], 0.125)
            if not no_in_dma:
                # het-0 slices of wk/wq land first so the preamble kproj and
                # qproj start as soon as ~0.25MB arrives instead of 1MB.
                nc.sync.dma_start(wk16[:, :, 0:128], wkT_t[:, :, 0:128])
                nc.sync.dma_start(xk16[:, :, 0:xsplit], xkT_t[:, :, 0:xsplit])
                nc.sync.dma_start(wk16[:, :, 128:512], wkT_t[:, :, 128:512])
                if SK > xsplit:
                    nc.sync.dma_start(xk16[:, :, xsplit:SK],
                                      xkT_t[:, :, xsplit:SK])
                nc.sync.dma_start(wv16[:], wvT_t)
                nc.sync.dma_start(wq16[:, :, 0:128], wqT_t[:, :, 0:128])
                nc.sync.dma_start(wq16[:, :, 128:512], wqT_t[:, :, 128:512])
                nc.sync.dma_start(x16[:, :, 0:1024], xT_t[:, :, 0:1024])
                nc.sync.dma_start(x16[:, :, 1024:2048], xT_t[:, :, 1024:2048])
                nc.sync.dma_start(wo16[:], woT_t)

            # ---- constants / small tiles
            mask_i = cpool.tile([P, ktp], I32)
            nc.gpsimd.dma_start(mask_i[:], mask_t)
            mask_f = cpool.tile([P, ktp], F32)
            nc.vector.tensor_copy(mask_f[:], mask_i[:])
            maskb = cpool.tile([P, ktp], F32)
            # (m - 1) * 10000 -> 0 for keep, -1e4 for masked/padding
            nc.vector.tensor_scalar(maskb[:], mask_f[:], -1.0, 10000.0, ADD, MUL)
            ones = cpool.tile([P, P], F16)
            nc.vector.memset(ones[:], 1.0)
            if padrep:
                # rows 0-64: only row 64 is ones — a K=65 stationary so the
                # denominator-broadcast matmul avoids the K<=64 half-rate mode
                ones65 = cpool.tile([65, 64], BF16)
                nc.vector.memset(ones65[0:64, :], 0.0)
                nc.vector.memset(ones65[64:65, :], 1.0)
            else:
                onesb = cpool.tile([1, 64], BF16)
                nc.vector.memset(onesb[:], 1.0)
            if not zero_bias:
                bq_sb = cpool.tile([P, NHPL], F32)
                nc.gpsimd.dma_start(bq_sb[:], bq.rearrange("(t p) -> p t", p=P))
                bk_sb = cpool.tile([P, NHPL], F32)
                nc.gpsimd.dma_start(bk_sb[:], bk.rearrange("(t p) -> p t", p=P))
                bv_sb = cpool.tile([1, HEL], F16)
                nc.gpsimd.dma_start(bv_sb[:], bv[None, :])

            vv = v16[:].rearrange("p t (h e) -> p t h e", e=65)
            nc.vector.memset(vv[:, :, :, 64:65], 1.0)

            KCHUNKS = [(o, min(512, SK - o)) for o in range(0, SK, 512)]

            def kproj_chunk(het, chunk):
                off, w = KCHUNKS[chunk]
                ps = pps.tile([P, 1024], F32, tag="sc", name="psk", bufs=SCB)
                pss = ps[:, 0:w]
                for dt in range(DT):
                    nc.tensor.matmul(
                        pss, wk16[:, dt, ts(het, P)],
                        xk16[:, dt, off:off + w],
                        start=(dt == 0), stop=(dt == DT - 1))
                if padk:
                    nc.gpsimd.memset(kTz[64:128, 0, het, off:off + w], 0.0)
                    nc.gpsimd.memset(kTz[0:64, 1, het, off:off + w], 0.0)
                    if zero_bias:
                        nc.vector.tensor_copy(
                            kTz[0:64, 0, het, off:off + w], pss[0:64, :])
                        nc.vector.tensor_copy(
                            kTz[64:128, 1, het, off:off + w], pss[64:128, :])
                    else:
                        nc.vector.tensor_tensor(
                            kTz[0:64, 0, het, off:off + w], pss[0:64, :],
                            bk_sb[0:64, het:het + 1].to_broadcast((64, w)),
                            ADD)
                        nc.vector.tensor_tensor(
                            kTz[64:128, 1, het, off:off + w], pss[64:128, :],
                            bk_sb[64:128, het:het + 1].to_broadcast((64, w)),
                            ADD)
                elif zero_bias:
                    nc.vector.tensor_copy(kT16[:, het, off:off + w], pss)
                else:
                    nc.vector.tensor_tensor(
                        kT16[:, het, off:off + w], pss,
                        bk_sb[:, het:het + 1].to_broadcast((P, w)), ADD)

            def vproj(kt):
                """V rows for key tile kt (all 8 heads) + bias + ones col."""
                ps = pps.tile([P, 1024], F32, tag="sc", name="psv", bufs=SCB)
                pss = ps[:, 0:512]
                for dt in range(DT):
                    nc.tensor.matmul(
                        pss, xk16[:, dt, ts(kt, P)],
                        wv16[:, dt, :],
                        start=(dt == 0),
                        stop=(zero_bias and dt == DT - 1))
                if not zero_bias:
                    nc.tensor.matmul(
                        pss, ones[0:1, 0:P], bv_sb[0:1, :],
                        start=False, stop=True)
                dst = vv[:, kt, :, 0:64]
                nc.vector.tensor_copy(
                    dst, pss.rearrange("p (h e) -> p h e", e=64))

            def qproj_chunk(hp, qh, qT, qch):
                ps = pps.tile([P, 1024], F32, tag="sc", name="psq", bufs=SCB)
                pss = ps[:, 0:512]
                for dt in range(DT):
                    nc.tensor.matmul(
                        pss, wq16[:, dt, ts(hp, P)],
                        x16[:, dt, qh * 1024 + qch * 512:qh * 1024 + (qch + 1) * 512],
                        start=(dt == 0), stop=(dt == DT - 1))
                if zero_bias:
                    nc.vector.tensor_copy(qT[:, ts(qch, 512)], pss)
                else:
                    nc.vector.tensor_tensor(
                        qT[:, ts(qch, 512)], pss,
                        bq_sb[:, hp:hp + 1].to_broadcast((P, 512)), ADD)

            def qproj(hp, qh, qT):
                for qch in range(2):
                    qproj_chunk(hp, qh, qT, qch)

            with (
                tc.tile_pool(name="qt", bufs=2) as qpool,
                tc.tile_pool(name="exp", bufs=ebufs) as epool,
                tc.tile_pool(name="fin", bufs=2 if depth2 else 1) as fpool,
                tc.tile_pool(name="ost", bufs=4) as opool,
            ):
                if stage >= 2:
                    for chunk in range(len(KCHUNKS)):
                        kproj_chunk(0, chunk)
                    for kt in range(ktp):
                        vproj(kt)
                    qT_tiles = {0: qpool.tile([P, 1024], F16, tag="qTn", name="qT0")}
                    qproj(0, 0, qT_tiles[0])
                if stage == 2:
                    for it in range(1, 8):
                        hp, qh = it >> 1, it & 1
                        if qh == 0:
                            for chunk in range(len(KCHUNKS)):
                                kproj_chunk(hp, chunk)
                        qT_tiles[it % 2] = qpool.tile([P, 1024], F16, name="qTn")
                        qproj(hp, qh, qT_tiles[it % 2])

                def scores_exp(hp, qT, qhh, kt):
                    """Scores for 512 queries x both head halves into ONE
                    2-bank PSUM tile (hh0 cols 0-511, hh1 cols 512-1023; the
                    two matmuls sit in different PE row groups and overlap),
                    then a single fused 1024-wide exp. Returns the exp tile."""
                    sc = pps.tile([P, 1024], F32, tag="sc", name="sc", bufs=SCB)
                    for hh in range(2):
                        if padk:
                            nc.tensor.matmul(
                                sc[:, ts(hh, 512)],
                                kTz[:, hh, hp, ts(kt, P)],
                                qT[:, ts(qhh, 512)],
                                start=True, stop=True)
                        else:
                            nc.tensor.matmul(
                                sc[:, ts(hh, 512)],
                                kT16[hh * 64:(hh + 1) * 64, hp, ts(kt, P)],
                                qT[hh * 64:(hh + 1) * 64, ts(qhh, 512)],
                                start=True, stop=True)
                    et = epool.tile([P, 1024], F16, name="et")
                    if tiny_exp:
                        nc.scalar.activation(
                            et[:, 496:528], sc[:, 496:528], Exp,
                            bias=maskb[:, kt:kt + 1], scale=SCALE)
                    else:
                        nc.scalar.activation(
                            et[:], sc[:], Exp,
                            bias=maskb[:, kt:kt + 1], scale=SCALE)
                    return et

                def av(hp, zt, kt, et):
                    for hh in range(2):
                        nc.tensor.matmul(
                            zt[hh][:],
                            vv[:, kt, hp * 2 + hh, 0:65],
                            et[:, ts(hh, 512)],
                            start=(kt == 0), stop=(kt == ktp - 1))

                def finalize(hp, qh, qhh, zt):
                    """Normalize z into mh^T columns for (qh, qhh)."""
                    zcd = fpool.tile([65, 2, 512], BF16, tag="zc")
                    with nc.allow_low_precision(reason="softmax"):
                        nc.vector.reciprocal(zcd[64:65, 0, :],
                                             zt[0][64:65, :])
                        nc.vector.reciprocal(zcd[64:65, 1, :],
                                             zt[1][64:65, :])
                        nc.vector.tensor_copy(zcd[0:64, 0, :],
                                              zt[0][0:64, :])
                        nc.vector.tensor_copy(zcd[0:64, 1, :],
                                              zt[1][0:64, :])
                    qcol = qh * 1024 + qhh * 512
                    for hh in range(2):
                        rep = pps.tile([64, 512], F32,
                                       tag=f"z{2 * qhh + hh}", name="rep")
                        nc.tensor.matmul(
                            rep[:], ones65[:, :],
                            zcd[0:65, hh, :], start=True, stop=True)
                        if hh == 0:
                            nc.vector.tensor_tensor(
                                mh16[0:64, hp, qcol:qcol + 512],
                                zcd[0:64, hh, :], rep[:], MUL)
                        else:
                            tmp = fpool.tile([64, 512], F16, tag="tmp")
                            nc.vector.tensor_tensor(
                                tmp[:], zcd[0:64, hh, :], rep[:], MUL)
                            nc.sync.dma_start(
                                mh16[64:128, hp, qcol:qcol + 512], tmp[:])

                def outproj_one(qt, nch):
                    ps = pps.tile([P, 1024], F32, tag="sc", name="pso",
                                  bufs=SCB)
                    pss = ps[:, 0:512]
                    for hp in range(NHPL):
                        nc.tensor.matmul(
                            pss, mh16[:, hp, ts(qt, P)],
                            wo16[:, hp, ts(nch, 512)],
                            start=(hp == 0),
                            stop=(hp == NHPL - 1))
                    ot = opool.tile([P, 512], F16)
                    if depth2 or (qt + nch) % 2 == 0:
                        nc.vector.tensor_copy(ot[:], pss)
                    else:
                        nc.scalar.copy(ot[:], pss)
                    if not no_out_dma:
                        nc.sync.dma_start(out_t[:, qt, ts(nch, 512)], ot[:])

                for it in range(8 if stage >= 3 else 0):
                    hp, qh = it >> 1, it & 1
                    qT = qT_tiles.pop(it % 2)

                    # Striped work for the next (hp, qh) iteration. With
                    # depth2, jobs are deferred to the start of the qhh=1
                    # block so they cover the qhh0-finalize + exp-latency
                    # window, while job-free stretches run a depth-2 exp
                    # prefetch (sc bufs=3) that hides the PE->ACT->PE
                    # round-trip latency.
                    jobs = []
                    if it + 1 < 8:
                        nhp, nqh = (it + 1) >> 1, (it + 1) & 1
                        if nqh == 1:
                            jobs = [("q", nhp, 1, 0), ("q", nhp, 1, 1)]
                            if hp + 1 < NHPL:
                                jobs += [("k", hp + 1, c)
                                         for c in range(1, len(KCHUNKS))]
                        else:
                            jobs = [("k", nhp, 0), ("q", nhp, 0, 0),
                                    ("q", nhp, 0, 1)]
                    else:
                        for qt in range(8):
                            for nch in range(2):
                                jobs.append(("o", qt, nch))

                    def do_job(job):
                        if job[0] == "k":
                            kproj_chunk(job[1], job[2])
                        elif job[0] == "q":
                            _, jhp, jqh, jch = job
                            if jch == 0:
                                qT_tiles[(it + 1) % 2] = qpool.tile(
                                    [P, 1024], F16, name="qTn")
                            qproj_chunk(jhp, jqh, qT_tiles[(it + 1) % 2], jch)
                        else:
                            outproj_one(job[1], job[2])

                    if depth2:
                        # A/B interleave: both qhh chains advance kt-by-kt,
                        # each chain's PE->ACT->PE latency hidden behind the
                        # other chain's matmuls. z0/z1 accumulate qhh=0,
                        # z2/z3 accumulate qhh=1 (sc bufs=2 + 4 z = 8 banks).
                        ztA = [pps.tile([65, 512], F32, tag=f"z{i}",
                                        name=f"z{i}") for i in range(2)]
                        ztB = [pps.tile([65, 512], F32, tag=f"z{2 + i}",
                                        name=f"z{2 + i}") for i in range(2)]
                        etA = [scores_exp(hp, qT, 0, 0)]
                        etB = [scores_exp(hp, qT, 1, 0)]
                        for kt in range(ktp):
                            if kt + 1 < ktp:
                                etA.append(scores_exp(hp, qT, 0, kt + 1))
                            av(hp, ztA, kt, etA.pop(0))
                            if kt + 1 < ktp:
                                etB.append(scores_exp(hp, qT, 1, kt + 1))
                            av(hp, ztB, kt, etB.pop(0))
                            if kt >= (1 if it == 0 else 0):
                                n = -(-len(jobs) // max(1, ktp - kt))
                                for _ in range(n):
                                    if jobs:
                                        do_job(jobs.pop(0))
                        for job in jobs:
                            do_job(job)
                        jobs = []
                        finalize(hp, qh, 0, ztA)
                        if it == 7:
                            for qt in range(8, 12):
                                for nch in range(2):
                                    outproj_one(qt, nch)
                        finalize(hp, qh, 1, ztB)
                        continue

                    for qhh in range(2):
                        if it == 7 and qhh == 1:
                            for qt in range(8, 12):
                                for nch in range(2):
                                    jobs.append(("o", qt, nch))

                        zt = [pps.tile([65, 512], F32, tag=f"z{i}",
                                       name=f"z{i}") for i in range(2)]
                        if not depth2:
                            n_slots = ((2 - qhh) * (ktp - 1)
                                       if ktp > 1 else 1)
                            per_kt = -(-len(jobs) // max(1, n_slots))
                            et = scores_exp(hp, qT, qhh, 0)
                            for kt in range(ktp):
                                nxt = (scores_exp(hp, qT, qhh, kt + 1)
                                       if kt + 1 < ktp else None)
                                av(hp, zt, kt, et)
                                et = nxt
                                if kt >= 1:
                                    for _ in range(per_kt):
                                        if not jobs:
                                            break
                                        do_job(jobs.pop(0))
                            if qhh == 1:
                                for job in jobs:
                                    do_job(job)
                                jobs = []
                        else:
                            etq = []
                            issued = 0
                            for kt in range(ktp):
                                want = kt + 2 + (0 if jobs else 1)
                                while issued < min(ktp, want):
                                    etq.append(scores_exp(hp, qT, qhh,
                                                          issued))
                                    issued += 1
                                av(hp, zt, kt, etq.pop(0))
                                rem = ktp - 1 - kt
                                if qhh == 0:
                                    # defer: run only what won't fit later
                                    while jobs and len(jobs) > rem + ktp:
                                        do_job(jobs.pop(0))
                                else:
                                    n = -(-len(jobs) // max(1, ktp - kt))
                                    for _ in range(n):
                                        if jobs:
                                            do_job(jobs.pop(0))
                            if qhh == 1:
                                for job in jobs:
                                    do_job(job)
                                jobs = []

                        # --- normalize z, write mh^T (denominator in row 64).
                        # Reciprocal runs straight off PSUM row 64 with a bf16
                        # output (wide exponent: no rescale); a PE ones-matmul
                        # broadcasts it across 64 partitions. Evacuation split
                        # DVE/ACT so the banks free without serializing.
                        if padrep:
                            # zcd rows 0-63 = z, row 64 = 1/den: one bf16
                            # tile so the broadcast matmul runs K=65 (full
                            # PE rate) with the ones65 stationary.
                            zcd = fpool.tile([65, 2, 512], BF16, tag="zc")
                            with nc.allow_low_precision(reason="softmax"):
                                nc.vector.reciprocal(zcd[64:65, 0, :],
                                                     zt[0][64:65, :])
                                nc.vector.reciprocal(zcd[64:65, 1, :],
                                                     zt[1][64:65, :])
                                nc.vector.tensor_copy(zcd[0:64, 0, :],
                                                      zt[0][0:64, :])
                                if depth2:
                                    nc.vector.tensor_copy(zcd[0:64, 1, :],
                                                          zt[1][0:64, :])
                                else:
                                    nc.scalar.copy(zcd[0:64, 1, :],
                                                   zt[1][0:64, :])
                        else:
                            den = fpool.tile([1, 2, 512], BF16, tag="den")
                            zc = fpool.tile([64, 2, 512], F32, tag="zc")
                            with nc.allow_low_precision(reason="softmax"):
                                nc.vector.reciprocal(den[0:1, 0, :],
                                                     zt[0][64:65, :])
                                nc.vector.reciprocal(den[0:1, 1, :],
                                                     zt[1][64:65, :])
                                nc.vector.tensor_copy(zc[:, 0, :],
                                                      zt[0][0:64, :])
                                nc.scalar.copy(zc[:, 1, :], zt[1][0:64, :])
                        qcol = qh * 1024 + qhh * 512
                        for hh in range(2):
                            rep = pps.tile([64, 512], F32, tag=f"z{hh}",
                                           name="rep")
                            if padrep:
                                nc.tensor.matmul(
                                    rep[:], ones65[:, :],
                                    zcd[0:65, hh, :], start=True, stop=True)
                                zsrc = zcd[0:64, hh, :]
                            else:
                                nc.tensor.matmul(
                                    rep[:], onesb[0:1, :],
                                    den[0:1, hh, :], start=True, stop=True)
                                zsrc = zc[:, hh, :]
                            if hh == 0:
                                nc.vector.tensor_tensor(
                                    mh16[0:64, hp, qcol:qcol + 512],
                                    zsrc, rep[:], MUL)
                            else:
                                tmp = fpool.tile([64, 512], F16, tag="tmp")
                                nc.vector.tensor_tensor(
                                    tmp[:], zsrc, rep[:], MUL)
                                nc.sync.dma_start(
                                    mh16[64:128, hp, qcol:qcol + 512], tmp[:])
                if stage >= 4:
                    for qt in range(12, 16):
                        for nch in range(2):
                            outproj_one(qt, nch)

    nc.compile()
    return nc


_NC_CACHE = {}


def _get_nc(zero_bias, ktp):
    key = (zero_bias, ktp)
    if key not in _NC_CACHE:
        _NC_CACHE[key] = build_nc(zero_bias=zero_bias, ktp=ktp)
    return _NC_CACHE[key]


def prep_in_maps(x, attention_mask, Wq, bq, Wk, bk, Wv, bv, Wo, bo,
                 use_bf16=False):
    """Build the per-core input maps (head-sharded: core = batch*2 + head-half).
    x and the weights are pre-cast to fp16; the key sequence is compacted to
    the unmasked positions (padded to a 128 multiple). Returns (in_maps, ktp).
    """
    if use_bf16:
        import ml_dtypes
        np_half = ml_dtypes.bfloat16
    else:
        np_half = np.float16
    x = np.asarray(x, dtype=np.float32)
    attention_mask = np.asarray(attention_mask, dtype=np.int32)
    B = x.shape[0]
    HE = NH * 64
    wqT = np.asarray(Wq).transpose(2, 0, 1).reshape(D, HE).astype(np_half)
    wkT = np.asarray(Wk).transpose(2, 0, 1).reshape(D, HE).astype(np_half)
    wvT = np.asarray(Wv).transpose(2, 0, 1).reshape(D, HE).astype(np_half)
    woT = np.asarray(Wo).T.astype(np_half)
    bqf = np.asarray(bq, dtype=np.float32).reshape(HE)
    bkf = np.asarray(bk, dtype=np.float32).reshape(HE)
    bvf = np.asarray(bv, dtype=np.float32).reshape(HE)

    idxs = [np.nonzero(attention_mask[b, 0])[0] for b in range(B)]
    ktp = max(1, -(-max(len(i) for i in idxs) // P))
    ktp = min(ktp, KT_FULL)
    SK = ktp * P

    xTs, xkTs, masks = [], [], []
    for b in range(B):
        xf = x[b].T.astype(np_half)
        xTs.append(np.ascontiguousarray(xf))
        nk = len(idxs[b])
        xk = np.zeros((D, SK), dtype=np_half)
        xk[:, :nk] = xf[:, idxs[b]]
        xkTs.append(xk)
        mc = np.zeros(SK, dtype=np.int32)
        mc[:nk] = 1
        masks.append(mc)

    in_maps = []
    for c in range(N_CORES):
        b, h2 = c // 2, c % 2
        sl = slice(h2 * HEL, (h2 + 1) * HEL)
        in_maps.append({
            "xT": xTs[b],
            "xkT": xkTs[b],
            "wqT": np.ascontiguousarray(wqT[:, sl]),
            "wkT": np.ascontiguousarray(wkT[:, sl]),
            "wvT": np.ascontiguousarray(wvT[:, sl]),
            "woT": np.ascontiguousarray(woT[sl, :]),
            "mask": masks[b],
            "bq": np.ascontiguousarray(bqf[sl]),
            "bk": np.ascontiguousarray(bkf[sl]),
            "bv": np.ascontiguousarray(bvf[sl]),
        })
    return in_maps, ktp


def kernel(x, attention_mask, Wq, bq, Wk, bk, Wv, bv, Wo, bo, trace=False):
    B = np.asarray(x).shape[0]
    in_maps, ktp = prep_in_maps(x, attention_mask, Wq, bq, Wk, bk, Wv, bv,
                                Wo, bo)

    zb = bool(np.all(np.asarray(bq) == 0) and np.all(np.asarray(bk) == 0)
              and np.all(np.asarray(bv) == 0))
    nc = _get_nc(zb, ktp)
    res = bass_utils.run_bass_kernel_spmd(
        nc, in_maps, core_ids=list(range(N_CORES)), trace=trace)
    bof = np.asarray(bo, dtype=np.float32).reshape(D)
    out = np.empty((B, S, D), dtype=np.float32)
    for b in range(B):
        out[b] = res.results[2 * b]["out"].astype(np.float32)
        out[b] += res.results[2 * b + 1]["out"].astype(np.float32)
        out[b] += bof
    kernel.last_result = res
    return out

